# revision 6
# baseline (speedup 1.0000x reference)
"""Trainium2 Bass kernel for NeuralGraphHidden (GNN message passing), v6.

v5 -> v6: degree select is eliminated entirely. Within each compact tile,
slot columns are grouped by atom degree into FIXED ranges (capacities =
max needed across cores, so the SPMD program is core-independent). The
dense stage runs one 3-matmul chain per non-empty range against just that
degree's weight block, so no masks, no DVE select chain, and tile 0
(packed with pure-modal-degree molecules) is a single range. Support DMAs
are split per consumer (bond rows ship as their own 64-descriptor tensor),
bond reduces + chunk2 assembly run on GpSimd, featT copies on Vector, so
ScalarE only triggers weight DMAs (no ACT_TABLE_LOAD stall).

See kernel_v2.py docstring for the core algorithm (active-atom compaction).
"""

import sys

sys.path.insert(0, "/opt/trn_rl_repo")

import numpy as np

B, A, D = 256, 128, 5
FA, FB, C = 256, 64, 256
F = FA + FB        # 320
FAUG = F + 1       # 321 (bias row)
NCORES = 8
BL = B // NCORES   # 32 molecules per core
TILES = 2          # compact tiles per core
CAP = 128          # compact slots per tile
BINW = FA + CAP    # per-bin blob width (256 atom cols + 128 gather cols)
NWARM = 22         # PE clock-gate warmup matmuls

_CACHE = {}


def _build_program(ranges, nbts, mod_di):
    """ranges: per tile, tuple of (base, width, weight_block_index).
    nbts: gather bin count per tile. mod_di: modal degree block index."""
    from contextlib import ExitStack

    import concourse.bass as bass
    import concourse.tile as tile
    from concourse import bacc, mybir

    f32 = mybir.dt.float32
    OP = mybir.AluOpType
    bf16 = mybir.dt.bfloat16

    ND = max(di for tr in ranges for (_, _, di) in tr) + 1
    NW = ND * C
    tbase = [0]
    for t in range(TILES):
        tbase.append(tbase[-1] + nbts[t] * BINW)
    IBW = tbase[-1]          # total input blob width

    nc = bacc.Bacc("TRN2", target_bir_lowering=False, debug=False,
                   num_devices=NCORES)

    inblob_d = nc.dram_tensor("inblob", [128, IBW], bf16,
                              kind="ExternalInput")
    # modal-degree weight block ships first; remaining blocks after
    wmod_d = nc.dram_tensor("wmod", [128, 3 * C], bf16,
                            kind="ExternalInput")
    NR = NW - C
    wrest_d = nc.dram_tensor("wrest", [128, 3 * NR], bf16,
                             kind="ExternalInput")
    bsum_d = nc.dram_tensor("bsum", [FB, TILES * CAP], bf16,
                            kind="ExternalInput")
    out_d = nc.dram_tensor("out", [CAP, TILES * C], f32,
                           kind="ExternalOutput")

    with tile.TileContext(nc) as tc, ExitStack() as ctx:
        consts = ctx.enter_context(tc.tile_pool(name="consts", bufs=1))
        pfeat = ctx.enter_context(tc.tile_pool(name="pfeat", bufs=2))
        ps_f = ctx.enter_context(
            tc.tile_pool(name="ps_f", bufs=2, space="PSUM"))
        ps_z = ctx.enter_context(
            tc.tile_pool(name="ps_z", bufs=2, space="PSUM"))

        # ---- PE warmup: dummy matmuls on a zeroed tile -------------------
        warm0 = consts.tile([128, 128], bf16)
        nc.vector.memset(warm0[:], 0.0)
        for i in range(NWARM):
            pw = ps_z.tile([128, 128], f32, tag="pw")
            nc.tensor.matmul(pw[:], warm0[:], warm0[:], start=True, stop=True)

        # ---- input DMAs --------------------------------------------------
        # tile-0 halves first on BOTH queues so its gather unblocks early
        inblob = consts.tile([128, IBW], bf16)
        for t in range(TILES):
            tw = nbts[t] * BINW
            hw = (nbts[t] // 2) * BINW if nbts[t] > 1 else tw
            pieces = ([(tbase[t], tbase[t] + hw),
                       (tbase[t] + hw, tbase[t] + tw)]
                      if hw < tw else [(tbase[t], tbase[t] + tw)])
            for pi, (lo, hi) in enumerate(pieces):
                eng = nc.sync if pi % 2 == 0 else nc.gpsimd
                eng.dma_start(out=inblob[:, lo:hi],
                              in_=inblob_d.ap()[:, lo:hi])
        bsum = consts.tile([FB, TILES * CAP], bf16)
        nc.scalar.dma_start(out=bsum[:], in_=bsum_d.ap()[:])
        wmod = consts.tile([128, 3 * C], bf16)
        nc.scalar.dma_start(out=wmod[:], in_=wmod_d.ap()[:])
        wrest = consts.tile([128, 3 * NR], bf16)
        nc.scalar.dma_start(out=wrest[:], in_=wrest_d.ap()[:])

        def wslice(chunk, di):
            if di == mod_di:
                return wmod[0:(FAUG - 256 if chunk == 2 else 128),
                            chunk * C:(chunk + 1) * C]
            ri = di if di < mod_di else di - 1
            return wrest[0:(FAUG - 256 if chunk == 2 else 128),
                         chunk * NR + ri * C:chunk * NR + (ri + 1) * C]

        out_sb = consts.tile([CAP, TILES * C], f32)
        nc.gpsimd.memset(out_sb[:], 0.0)

        # chunk2 = [bond sums; ones] [FB+1, CAP] per tile, built up front
        chunk2s = []
        for t in range(TILES):
            chunk2 = pfeat.tile([FB + 1, CAP], bf16, tag=f"c2_{t}")
            nc.gpsimd.tensor_copy(chunk2[0:FB, :],
                                  bsum[:, t * CAP:(t + 1) * CAP])
            nc.gpsimd.memset(chunk2[FB:FB + 1, :], 1.0)
            chunk2s.append(chunk2)

        # ---- per-tile pipeline -------------------------------------------
        for t in range(TILES):
            tb = tbase[t]
            # gather straight into featT layout: [feat chunk, slots]
            fps = []
            for k in range(2):
                fp = ps_f.tile([128, CAP], f32, tag=f"fp{k}")
                for bi in range(nbts[t]):
                    nc.tensor.matmul(
                        fp[:],
                        inblob[:, tb + bi * BINW + k * 128:
                               tb + bi * BINW + (k + 1) * 128],
                        inblob[:, tb + bi * BINW + FA:
                               tb + (bi + 1) * BINW],
                        start=(bi == 0), stop=(bi == nbts[t] - 1))
                fps.append(fp)
            featT = pfeat.tile([128, 2 * CAP], bf16, tag="ftT")
            with nc.allow_low_precision(reason="bf16 feat, tol 2e-2"):
                for k in range(2):
                    nc.vector.tensor_copy(featT[:, k * CAP:(k + 1) * CAP],
                                          fps[k][:])

            chunk2 = chunk2s[t]
            # dense: one 3-matmul chain per degree range of this tile
            pz = ps_z.tile([CAP, 512], f32, tag="pz")
            for (r0, rw, di) in ranges[t]:
                nc.tensor.matmul(pz[r0:r0 + rw, 0:C],
                                 featT[:, r0:r0 + rw],
                                 wslice(0, di), start=True, stop=False)
                nc.tensor.matmul(pz[r0:r0 + rw, 0:C],
                                 featT[:, CAP + r0:CAP + r0 + rw],
                                 wslice(1, di), start=False, stop=False)
                nc.tensor.matmul(pz[r0:r0 + rw, 0:C],
                                 chunk2[:, r0:r0 + rw],
                                 wslice(2, di), start=False, stop=True)
            tot = ranges[t][-1][0] + ranges[t][-1][1]
            nc.vector.tensor_scalar(out_sb[0:tot, t * C:(t + 1) * C],
                                    pz[0:tot, 0:C], 0.0, None, OP.max)
            nc.scalar.dma_start(out=out_d.ap()[:, t * C:(t + 1) * C],
                                in_=out_sb[:, t * C:(t + 1) * C])

    nc.compile()
    return nc


def _get_nc(ranges, nbts, mod_di):
    key = ("nc", tuple(tuple(tr) for tr in ranges), tuple(nbts), mod_di)
    if key not in _CACHE:
        _CACHE[key] = _build_program(ranges, nbts, mod_di)
    return _CACHE[key]


def _prep(atoms, bonds, edges, W, b):
    """Host-side: degree analysis, tile/bin packing, blob assembly."""
    import ml_dtypes

    bf16 = ml_dtypes.bfloat16
    atoms = np.asarray(atoms, dtype=np.float32)
    bonds = np.asarray(bonds, dtype=np.float32)
    edges = np.asarray(edges)
    W = np.asarray(W, dtype=np.float32)
    b = np.asarray(b, dtype=np.float32)

    deg = (edges != -1).sum(axis=-1)                      # (B, A)
    active = deg <= D - 1                                 # (B, A)
    deg_list = sorted(int(d) for d in np.unique(deg[active]))
    if not deg_list:
        deg_list = [D - 1]
    ND = len(deg_list)
    dpos = {d: i for i, d in enumerate(deg_list)}
    dcounts = {d: int((deg[active] == d).sum()) for d in deg_list}
    dmod = max(deg_list, key=lambda d: dcounts[d])

    waug = np.concatenate([W, b[:, None, :]], axis=1)     # (D, FAUG, C)
    waug = waug[deg_list].astype(bf16)                    # (ND, FAUG, C)
    NW = ND * C

    # ---- pass 1: per-core packing ------------------------------------
    cores = []
    need = np.zeros((NCORES, TILES, ND), dtype=int)  # slots per (t, deg)
    for c in range(NCORES):
        gm0 = c * BL
        acts, refs, refpos, pure = [], [], [], []
        for m in range(BL):
            gm = gm0 + m
            aidx = np.nonzero(active[gm])[0]
            acts.append(aidx)
            ra = set(aidx.tolist())
            for a in aidx:
                for e in edges[gm, a]:
                    if e >= 0:
                        ra.add(int(e))
            ra = sorted(ra)
            refs.append(ra)
            refpos.append({a: i for i, a in enumerate(ra)})
            pure.append(all(deg[gm, a] == dmod for a in aidx))

        slots = [0, 0]
        rsums = [0, 0]
        tmem = [[], []]

        def fits(t, m):
            return (slots[t] + len(acts[m]) <= CAP and
                    rsums[t] + len(refs[m]) <= 580)

        order = sorted(range(BL), key=lambda m: -len(refs[m]))
        for m in order:
            if len(acts[m]) == 0:
                continue
            if not pure[m]:
                if not fits(1, m):
                    raise RuntimeError(f"core {c}: tile packing failed")
                t = 1
            else:
                # fill tile 0 first so tile 1 keeps few slots per degree
                if fits(0, m):
                    t = 0
                elif fits(1, m):
                    t = 1
                else:
                    raise RuntimeError(f"core {c}: tile packing failed")
            tmem[t].append(m)
            slots[t] += len(acts[m])
            rsums[t] += len(refs[m])
            for a in acts[m]:
                need[c, t, dpos[int(deg[gm0 + m, a])]] += 1
        cores.append((acts, refs, refpos, tmem))

    # fixed per-(tile, degree) range capacities, shared across cores
    # PE matmul output BASE partitions may only be 0, 32 or 64. Place
    # small ranges first at 0/32, the big (modal) range last at the next
    # free base with room to grow to 128.
    caps = need.max(axis=0)                               # (TILES, ND)
    ranges = []
    for t in range(TILES):
        present = [di for di in range(ND) if caps[t, di] > 0]
        present.sort(key=lambda di: (int(caps[t, di]), deg_list[di]))
        assert len(present) <= 3, "too many degree ranges per tile"
        bases = [0, 32, 64][:len(present)]
        if present:
            bases[-1] = min(64, bases[-1])
        tr = []
        for i, di in enumerate(present):
            b0 = bases[i]
            lim = bases[i + 1] if i + 1 < len(present) else CAP
            w = int(caps[t, di])
            assert b0 + w <= lim, "range capacity overflow"
            tr.append((b0, w, di))
        # big range may span from its base to CAP
        ranges.append(tuple(tr))

    # bins per tile (first-fit decreasing); per-tile bin counts
    binned = []
    nbts = [1] * TILES
    for c in range(NCORES):
        acts, refs, refpos, tmem = cores[c]
        tb = []
        for t in range(TILES):
            bins = []
            for m in sorted(tmem[t], key=lambda m: -len(refs[m])):
                for bn in bins:
                    if bn[0] + len(refs[m]) <= 128:
                        bn[0] += len(refs[m])
                        bn[1].append(m)
                        break
                else:
                    bins.append([len(refs[m]), [m]])
            tb.append([bn[1] for bn in bins])
            nbts[t] = max(nbts[t], len(bins))
        binned.append(tb)
    tbase = [0]
    for t in range(TILES):
        tbase.append(tbase[-1] + nbts[t] * BINW)
    IBW = tbase[-1]

    rbase = {}  # (t, di) -> column base
    for t in range(TILES):
        for (b0, w, di) in ranges[t]:
            rbase[(t, di)] = b0

    in_maps = []
    scatter = []
    for c in range(NCORES):
        gm0 = c * BL
        acts, refs, refpos, tmem = cores[c]
        inblob = np.zeros((128, IBW), dtype=np.float32)
        bondsrc = np.zeros((FB, TILES * CAP), dtype=np.float32)
        sc_flat, sc_m, sc_a = [], [], []
        for t in range(TILES):
            cur = {di: 0 for di in range(ND)}
            for k, mols in enumerate(binned[c][t]):
                base = tbase[t] + k * BINW
                r = 0
                for m in mols:
                    gm = gm0 + m
                    ra = refs[m]
                    L = len(ra)
                    inblob[r:r + L, base:base + FA] = atoms[gm, ra]
                    for a in acts[m]:
                        di = dpos[int(deg[gm, a])]
                        col = rbase[(t, di)] + cur[di]
                        cur[di] += 1
                        inblob[r + refpos[m][a], base + FA + col] += 1.0
                        for dd in range(D):
                            e = edges[gm, a, dd]
                            if e >= 0:
                                inblob[r + refpos[m][int(e)],
                                       base + FA + col] += 1.0
                        bondsrc[:, t * CAP + col] = bonds[gm, a].sum(0)
                        sc_flat.append(t * CAP + col)
                        sc_m.append(gm)
                        sc_a.append(int(a))
                    r += L
        mod_di = dpos[dmod]
        rest = [di for di in range(ND) if di != mod_di]
        wmod = np.zeros((128, 3 * C), dtype=np.float32)
        wrest = np.zeros((128, 3 * (NW - C)), dtype=np.float32)
        NR = NW - C
        for chunk, (r0, r1) in enumerate([(0, 128), (128, 256),
                                          (256, FAUG)]):
            wmod[0:r1 - r0, chunk * C:(chunk + 1) * C] = \
                waug[mod_di, r0:r1, :]
            for j, di in enumerate(rest):
                wrest[0:r1 - r0, chunk * NR + j * C:
                      chunk * NR + (j + 1) * C] = waug[di, r0:r1, :]
        scatter.append((np.asarray(sc_flat), np.asarray(sc_m),
                        np.asarray(sc_a)))
        in_maps.append({
            "inblob": inblob.astype(bf16),
            "wmod": wmod.astype(bf16),
            "wrest": wrest.astype(bf16),
            "bsum": bondsrc.astype(bf16),
        })
    return ranges, nbts, dpos[dmod], in_maps, scatter


def run_sharded(atoms, bonds, edges, W, b, trace=False):
    """Run on the 8 NeuronCores; returns (output, BassKernelResults)."""
    from concourse.bass_utils import run_bass_kernel_spmd

    ranges, nbts, mod_di, in_maps, scatter = _prep(atoms, bonds, edges,
                                                   W, b)
    nc = _get_nc(ranges, nbts, mod_di)
    res = run_bass_kernel_spmd(nc, in_maps, list(range(NCORES)), trace=trace)
    out = np.zeros((B, A, C), dtype=np.float32)
    for c in range(NCORES):
        sc_flat, sc_m, sc_a = scatter[c]
        oc = res.results[c]["out"]
        t_idx = sc_flat // CAP
        s_idx = sc_flat % CAP
        out[sc_m, sc_a] = oc[s_idx[:, None], (t_idx[:, None] * C +
                                              np.arange(C)[None, :])]
    return out, res


def kernel(atoms, bonds, edges, W, b):
    out, _ = run_sharded(atoms, bonds, edges, W, b)
    return out
